# revision 39
# baseline (speedup 1.0000x reference)
"""Trainium2 Bass kernel for nn_AttentionSACModel (sparse_attention).

Data-parallel across 8 NeuronCores: obs sharded along batch, params replicated.
On-device layout keeps batch on the matmul free dim (activations stored
feature-major / transposed); all host<->device layout changes happen in numpy.

Design (refactored from the 489us baseline; ~352us traced):
- ctx uses v-linearity: ctx = Wv.T @ (sum_n alpha_n * z_n); the weighted sum
  accumulates through PSUM via per-n Wv matmuls (PE) instead of per-n V tiles
  (kills 160 scalar copies, the gpsimd add tree and the big vector reduces).
- paired activations: z-prelu and energy-tanh read 2 PSUM banks in one
  activation op ([126, 1024]) to amortize the ~400ns fixed access latency.
- softmax denominator: EG groups pre-summed on gpsimd (SBUF bf16), then one
  matmul with the head-selection matrix + vector reciprocal, emitted at the
  E-phase tail so the ctx phase never stalls on it.
- 3-stage stagger inside each tile (z@c, k+q@c-2, scores@c-4) plus ctx/head
  steps of the previous tile interleaved between, split so the PE never sits
  behind a same-iteration scalar dependency; long back-to-back matmul runs
  let the PE p-state ramp toward 2.4 GHz (512-col bf16 matmul ~215ns).
- PSUM budget (8 banks): z-pair 2, kq-pair 2, scores 1, alpha-bcast 1,
  ctx-accum 1, small/head rotating 1.

Notes: fp8 DoubleRow, custom-DVE ops (reciprocal_approx_*) and
partition_broadcast all crash this image's walrus backend — do not use.
"""
import sys

if "/opt/trn_rl_repo" not in sys.path:
    sys.path.insert(0, "/opt/trn_rl_repo")

import numpy as np
import ml_dtypes
_bf16np = ml_dtypes.bfloat16

OWN_DIM = 7
INT_DIM = 7
N_INTR = 20
H = 3
D = 42
TOT = H * D            # 126
ATTN = 128
HID = 256
NOUT = 4
B = 32768
N_CORES = 8
BC = B // N_CORES      # 4096 rows per core
NB = 512               # batch tile (matmul free dim)
NT = BC // NB          # 8 tiles per core
ALPHA = 0.2            # leaky relu slope

_BUILT = {}


def _build_nc():
    import concourse.bacc as bacc
    import concourse.tile as tile
    from concourse import mybir

    f32 = mybir.dt.float32
    f32r = mybir.dt.float32r
    bf16 = mybir.dt.bfloat16
    AF = mybir.ActivationFunctionType
    ALU = mybir.AluOpType

    nc = bacc.Bacc()

    # ---- DRAM I/O ----
    xo_d = nc.dram_tensor("xo", [OWN_DIM, BC], bf16, kind="ExternalInput")
    xa_d = nc.dram_tensor("xa", [126, BC], bf16, kind="ExternalInput")    # interactors 0..17
    xb_d = nc.dram_tensor("xb", [14, BC], bf16, kind="ExternalInput")     # interactors 18,19
    wia_d = nc.dram_tensor("wia", [126, 18 * 126], bf16, kind="ExternalInput")
    wib_d = nc.dram_tensor("wib", [14, 2 * 126], bf16, kind="ExternalInput")
    wo_d = nc.dram_tensor("wo", [7, 126], bf16, kind="ExternalInput")
    wq_d = nc.dram_tensor("wqb", [126, 126], bf16, kind="ExternalInput")
    wk_d = nc.dram_tensor("wkb", [126, 126], bf16, kind="ExternalInput")
    wv_d = nc.dram_tensor("wvb", [126, 126], bf16, kind="ExternalInput")
    va_d = nc.dram_tensor("va32", [126, 32], bf16, kind="ExternalInput")
    ds_d = nc.dram_tensor("densel", [128, 3], bf16, kind="ExternalInput")
    eb_d = nc.dram_tensor("ebcsel", [128, 4 * 126], bf16, kind="ExternalInput")
    rb_d = nc.dram_tensor("rbc", [3, 126], f32r, kind="ExternalInput")
    wat_d = nc.dram_tensor("wat", [126, 128], f32r, kind="ExternalInput")
    wop_d = nc.dram_tensor("wop", [126, 128], bf16, kind="ExternalInput")
    wh1_d = nc.dram_tensor("wh1r", [128, 512], f32r, kind="ExternalInput")
    wh2_d = nc.dram_tensor("wh2r", [128, 512], f32r, kind="ExternalInput")
    wout_d = nc.dram_tensor("woutr", [128, 8], f32r, kind="ExternalInput")
    bown_d = nc.dram_tensor("bown", [126, 1], f32, kind="ExternalInput")
    bint_d = nc.dram_tensor("bint", [126, 1], f32, kind="ExternalInput")
    bat_d = nc.dram_tensor("bat", [128, 1], f32, kind="ExternalInput")
    bop_d = nc.dram_tensor("bop", [128, 1], f32, kind="ExternalInput")
    bh1_d = nc.dram_tensor("bh1", [128, 2], f32, kind="ExternalInput")
    bh2_d = nc.dram_tensor("bh2", [128, 2], f32, kind="ExternalInput")
    bout_d = nc.dram_tensor("bout", [4, 1], f32, kind="ExternalInput")
    out_d = nc.dram_tensor("outT", [NOUT, BC], f32, kind="ExternalOutput")

    with tile.TileContext(nc) as tc:
        with tc.tile_pool(name="const", bufs=1) as cst, \
             tc.tile_pool(name="px", bufs=3) as px, \
             tc.tile_pool(name="pzt", bufs=2) as pzt, \
             tc.tile_pool(name="pen", bufs=4) as pen, \
             tc.tile_pool(name="peg", bufs=2) as peg, \
             tc.tile_pool(name="pn", bufs=8) as pnp, \
             tc.tile_pool(name="ph", bufs=2) as ph, \
             tc.tile_pool(name="pz", bufs=1, space="PSUM") as ppz, \
             tc.tile_pool(name="pk", bufs=1, space="PSUM") as ppk, \
             tc.tile_pool(name="ps", bufs=1, space="PSUM") as pps, \
             tc.tile_pool(name="pb", bufs=1, space="PSUM") as ppb, \
             tc.tile_pool(name="pc", bufs=1, space="PSUM") as ppc, \
             tc.tile_pool(name="sm", bufs=1, space="PSUM") as psm:

            # ---- constants ----
            WiA = cst.tile([126, 18 * 126], bf16)
            WiB = cst.tile([14, 2 * 126], bf16)
            Wo = cst.tile([7, 126], bf16)
            Wq = cst.tile([126, 126], bf16)
            Wk = cst.tile([126, 126], bf16)
            Wv = cst.tile([126, 126], bf16)
            Va = cst.tile([126, 32], bf16)
            Ds = cst.tile([128, 3], bf16)
            Eb = cst.tile([128, 4 * 126], bf16)
            Rb = cst.tile([3, 126], f32r)
            Wat = cst.tile([126, 128], f32r)
            Wop = cst.tile([126, 128], bf16)
            WH1 = cst.tile([128, 512], f32r)
            WH2 = cst.tile([128, 512], f32r)
            WOUT = cst.tile([128, 8], f32r)
            Bown = cst.tile([126, 1], f32)
            Bint = cst.tile([126, 1], f32)
            Bat = cst.tile([128, 1], f32)
            Bop = cst.tile([128, 1], f32)
            BH1 = cst.tile([128, 2], f32)
            BH2 = cst.tile([128, 2], f32)
            Bout = cst.tile([4, 1], f32)
            for t_sb, t_dr in [(WiA, wia_d), (Wo, wo_d), (Bown, bown_d),
                               (Bint, bint_d), (WiB, wib_d), (Wk, wk_d),
                               (Wq, wq_d), (Va, va_d), (Ds, ds_d)]:
                nc.sync.dma_start(out=t_sb, in_=t_dr[:, :])

            def load_late_consts():
                for t_sb, t_dr in [(Eb, eb_d), (Rb, rb_d),
                                   (Wv, wv_d), (Wat, wat_d), (Wop, wop_d),
                                   (WH1, wh1_d), (WH2, wh2_d), (WOUT, wout_d),
                                   (Bat, bat_d), (Bop, bop_d), (BH1, bh1_d),
                                   (BH2, bh2_d), (Bout, bout_d)]:
                    nc.scalar.dma_start(out=t_sb, in_=t_dr[:, :])

            with nc.allow_low_precision(reason="bf16 intermediates; final accums f32"):
                state = {}

                def load_x(t):
                    bs = t * NB
                    XO = px.tile([OWN_DIM, NB], bf16, tag="xo", name="XO")
                    XA = px.tile([126, NB], bf16, tag="xa", name="XA")
                    XB = px.tile([14, NB], bf16, tag="xb", name="XB")
                    nc.sync.dma_start(out=XO, in_=xo_d[:, bs:bs + NB])
                    nc.sync.dma_start(out=XA, in_=xa_d[:, bs:bs + NB])
                    nc.sync.dma_start(out=XB, in_=xb_d[:, bs:bs + NB])
                    state[t] = {"X": (XO, XA, XB)}

                def e_pre(t):
                    """own embed for tile t (sm bank use #1 of the cycle)"""
                    XO = state[t]["X"][0]
                    PO = psm.tile([128, NB], f32, tag="sm", name="PO")
                    nc.tensor.matmul(PO[0:126, :], Wo, XO)
                    OWN = ph.tile([126, NB], bf16, tag="own", name="OWN")
                    nc.scalar.activation(OWN, PO[0:126, :], AF.Prelu,
                                         bias=Bown, alpha=ALPHA)
                    ZT = pzt.tile([126, N_INTR, NB], bf16, tag="zt", name="ZT")
                    EG = peg.tile([128, 5, NB], bf16, tag="eg", name="EG")
                    st = state[t]
                    st.update({"OWN": OWN, "ZT": ZT, "EG": EG})

                def z_mm(t, c):
                    """embed matmuls for chunk c (n=2c, 2c+1) -> pz pair bank"""
                    st = state[t]
                    XA, XB = st["X"][1], st["X"][2]
                    PZ = ppz.tile([126, 2, NB], f32, tag="pz", name="PZ")
                    for i in range(2):
                        n = 2 * c + i
                        if n < 18:
                            nc.tensor.matmul(PZ[:, i, :],
                                             WiA[:, n * 126:(n + 1) * 126], XA)
                        else:
                            nc.tensor.matmul(PZ[:, i, :],
                                             WiB[:, (n - 18) * 126:(n - 17) * 126], XB)
                    st[("PZ", c)] = PZ

                def z_act(t, c):
                    st = state[t]
                    nc.scalar.activation(st["ZT"][:, 2 * c:2 * c + 2, :],
                                         st.pop(("PZ", c)),
                                         AF.Prelu, bias=Bint, alpha=ALPHA)

                def kq_mm(t, c):
                    st = state[t]
                    ZT, OWN = st["ZT"], st["OWN"]
                    PK = ppk.tile([126, 2, NB], f32, tag="pk", name="PK")
                    for i in range(2):
                        n = 2 * c + i
                        nc.tensor.matmul(PK[:, i, :], Wk, ZT[:, n, :],
                                         start=True, stop=False)
                        nc.tensor.matmul(PK[:, i, :], Wq, OWN,
                                         start=False, stop=True)
                    st[("PK", c)] = PK

                def tanh_act(t, c):
                    st = state[t]
                    EN = pen.tile([126, 2, NB], bf16, tag="en", name="EN")
                    nc.scalar.activation(EN, st.pop(("PK", c)), AF.Tanh)
                    st[("EN", c)] = EN

                def score_mm(t, c):
                    st = state[t]
                    EN = st.pop(("EN", c))
                    for i in range(2):
                        n = 2 * c + i
                        j = n % 4
                        if j == 0:
                            st["PS"] = pps.tile([128, NB], f32, tag="ps", name="PS")
                        nc.tensor.matmul(st["PS"][32 * j:32 * (j + 1), :], Va,
                                         EN[:, i, :], tile_position=(0, 32 * j))

                def exp_act(t, g):
                    st = state[t]
                    nc.scalar.activation(st["EG"][:, g, :], st["PS"], AF.Exp)

                def egsum(t):
                    """denominator pre-sum on gpsimd (SBUF bf16 only)"""
                    st = state[t]
                    EG = st["EG"]
                    s01 = ph.tile([128, NB], bf16, tag="es0", name="ES0")
                    s23 = ph.tile([128, NB], bf16, tag="es1", name="ES1")
                    s03 = ph.tile([128, NB], bf16, tag="es2", name="ES2")
                    EGS = ph.tile([128, NB], bf16, tag="egs", name="EGS")
                    nc.gpsimd.tensor_tensor(out=s01, in0=EG[:, 0, :], in1=EG[:, 1, :], op=ALU.add)
                    nc.gpsimd.tensor_tensor(out=s23, in0=EG[:, 2, :], in1=EG[:, 3, :], op=ALU.add)
                    nc.gpsimd.tensor_tensor(out=s03, in0=s01, in1=s23, op=ALU.add)
                    nc.gpsimd.tensor_tensor(out=EGS, in0=s03, in1=EG[:, 4, :], op=ALU.add)
                    st["EGS"] = EGS

                def denom(t):
                    """softmax denominator + reciprocal, emitted at E-phase
                    tail so the ctx phase never stalls on the reciprocal"""
                    st = state[t]
                    PD = psm.tile([128, NB], f32, tag="sm", name="PD")
                    nc.tensor.matmul(PD[0:3, :], Ds, st["EGS"])
                    RD = ph.tile([3, NB], f32r, tag="rd", name="RD")
                    nc.vector.reciprocal(RD, PD[0:3, :])
                    st["RD"] = RD

                def build_c_steps(t):
                    """ctx + head for tile t, as a list of interleavable steps"""
                    st = state[t]
                    bs = t * NB

                    def c_rbcast():
                        PR = psm.tile([128, NB], f32, tag="sm", name="PR")
                        nc.tensor.matmul(PR[0:126, :], Rb, st["RD"])
                        PRs = ph.tile([126, NB], f32, tag="prs", name="PRs")
                        nc.vector.tensor_scalar_mul(PRs, PR[0:126, :], 1.0)
                        st["PR"] = PRs

                    def mk_cna(n):
                        def s():
                            g, j = n // 4, n % 4
                            PEb = ppb.tile([126, NB], f32, tag="pb", name="PEb")
                            nc.tensor.matmul(PEb, Eb[:, j * 126:(j + 1) * 126],
                                             st["EG"][:, g, :])
                            PN = pnp.tile([126, NB], bf16, tag="pn", name="PN")
                            nc.vector.tensor_tensor(out=PN, in0=PEb,
                                                    in1=st["ZT"][:, n, :], op=ALU.mult)
                            st[("PN", n)] = PN
                        return s

                    def mk_cnb(n):
                        def s():
                            if n == 0:
                                st["CTXV"] = ppc.tile([126, NB], f32, tag="pc",
                                                      name="CTXV")
                            nc.tensor.matmul(st["CTXV"], Wv, st.pop(("PN", n)),
                                             start=(n == 0), stop=(n == 19))
                        return s

                    def c_norm():
                        CTX = ph.tile([126, NB], f32r, tag="ctx", name="CTX")
                        nc.vector.tensor_tensor(out=CTX, in0=st["CTXV"],
                                                in1=st["PR"], op=ALU.mult)
                        st["CTX"] = CTX

                    def c_att():
                        PH = psm.tile([128, NB], f32, tag="sm", name="PH")
                        nc.tensor.matmul(PH, Wat, st["CTX"])
                        st["ATT"] = ph.tile([128, NB], f32r, tag="att", name="ATT")
                        nc.scalar.activation(st["ATT"], PH, AF.Tanh, bias=Bat)

                    def c_ownp():
                        PH = psm.tile([128, NB], f32, tag="sm", name="PH2")
                        nc.tensor.matmul(PH, Wop, st["OWN"])
                        st["OWV"] = ph.tile([128, NB], f32r, tag="owv", name="OWV")
                        nc.scalar.activation(st["OWV"], PH, AF.Tanh, bias=Bop)

                    def mk_h1(mh):
                        def s():
                            PHh = psm.tile([128, NB], f32, tag="sm", name="PHh")
                            nc.tensor.matmul(PHh, WH1[:, mh * 128:(mh + 1) * 128],
                                             st["OWV"], start=True, stop=False)
                            nc.tensor.matmul(PHh, WH1[:, 256 + mh * 128:256 + (mh + 1) * 128],
                                             st["ATT"], start=False, stop=True)
                            st[f"H1{mh}"] = ph.tile([128, NB], f32r, tag=f"h1{mh}", name="H1")
                            nc.scalar.activation(st[f"H1{mh}"], PHh, AF.Prelu,
                                                 bias=BH1[:, mh:mh + 1], alpha=ALPHA)
                        return s

                    def mk_h2(mh):
                        def s():
                            PHh = psm.tile([128, NB], f32, tag="sm", name="PHh2")
                            nc.tensor.matmul(PHh, WH2[:, mh * 128:(mh + 1) * 128],
                                             st["H10"], start=True, stop=False)
                            nc.tensor.matmul(PHh, WH2[:, 256 + mh * 128:256 + (mh + 1) * 128],
                                             st["H11"], start=False, stop=True)
                            st[f"H2{mh}"] = ph.tile([128, NB], f32r, tag=f"h2{mh}", name="H2")
                            nc.scalar.activation(st[f"H2{mh}"], PHh, AF.Prelu,
                                                 bias=BH2[:, mh:mh + 1], alpha=ALPHA)
                        return s

                    def c_out():
                        PO4 = psm.tile([128, NB], f32, tag="sm", name="PO4")
                        nc.tensor.matmul(PO4[0:4, :], WOUT[:, 0:4], st["H20"],
                                         start=True, stop=False)
                        nc.tensor.matmul(PO4[0:4, :], WOUT[:, 4:8], st["H21"],
                                         start=False, stop=True)
                        OT = ph.tile([4, NB], f32, tag="ot", name="OT")
                        nc.vector.tensor_scalar_add(OT, PO4[0:4, :], Bout)
                        nc.sync.dma_start(out=out_d[:, bs:bs + NB], in_=OT)
                        del state[t]

                    cns = [c_rbcast, mk_cna(0), mk_cna(1)]
                    for n in range(2, N_INTR):
                        cns += [mk_cnb(n - 2), mk_cna(n)]
                    cns += [mk_cnb(N_INTR - 2), mk_cnb(N_INTR - 1)]
                    return (cns + [c_norm, c_att, c_ownp, mk_h1(0), mk_h1(1),
                                   mk_h2(0), mk_h2(1), c_out])

                def emit_tile(t, csteps):
                    """stage E of tile t interleaved with ctx/head steps of t-1.

                    3-stage stagger inside E: z matmuls of chunk c, k+q of c-1,
                    scores of c-2 — keeps PE fed while scalar drains PSUM."""
                    ci = 0

                    def c_run(k):
                        nonlocal ci
                        for _ in range(k):
                            if ci < len(csteps):
                                csteps[ci]()
                                ci += 1

                    e_pre(t)
                    NCH = N_INTR // 2
                    for c in range(NCH + 4):
                        if c < NCH:
                            z_mm(t, c)
                        c_run(1)
                        if 1 <= c < NCH + 1:
                            z_act(t, c - 1)
                        c_run(1)
                        if 2 <= c < NCH + 2:
                            kq_mm(t, c - 2)
                        c_run(1)
                        if 3 <= c < NCH + 3:
                            tanh_act(t, c - 3)
                        c_run(1)
                        if 4 <= c < NCH + 4:
                            score_mm(t, c - 4)
                            if (c - 4) % 2 == 1:
                                exp_act(t, (c - 4) // 2)
                    egsum(t)
                    denom(t)
                    c_run(len(csteps))

                # ---- software pipeline over tiles ----
                load_x(0)
                load_x(1)
                emit_tile(0, [])
                load_late_consts()
                for t in range(1, NT):
                    if t + 1 < NT:
                        load_x(t + 1)
                    emit_tile(t, build_c_steps(t - 1))
                for s in build_c_steps(NT - 1):
                    s()

    nc.compile()
    return nc


def _host_prep(inputs):
    """Build per-core input maps (numpy only)."""
    obs = np.ascontiguousarray(inputs["obs"], dtype=np.float32)
    w_own = np.asarray(inputs["w_own"], np.float32)
    w_int = np.asarray(inputs["w_int"], np.float32)
    wq = np.asarray(inputs["wq"], np.float32)
    wk = np.asarray(inputs["wk"], np.float32)
    wv = np.asarray(inputs["wv"], np.float32)
    v_att = np.asarray(inputs["v_att"], np.float32)
    w_attn = np.asarray(inputs["w_attn"], np.float32)
    w_ownp = np.asarray(inputs["w_ownp"], np.float32)
    w_h1 = np.asarray(inputs["w_h1"], np.float32)
    w_h2 = np.asarray(inputs["w_h2"], np.float32)
    w_out = np.asarray(inputs["w_out"], np.float32)

    def blockdiag(w):  # [H, D, D] -> [126, 126]
        out = np.zeros((TOT, TOT), np.float32)
        for h in range(H):
            out[h * D:(h + 1) * D, h * D:(h + 1) * D] = w[h]
        return out

    wia = np.zeros((126, 18 * 126), np.float32)
    for n in range(18):
        wia[7 * n:7 * n + 7, n * 126:(n + 1) * 126] = w_int
    wib = np.zeros((14, 2 * 126), np.float32)
    for n in range(2):
        wib[7 * n:7 * n + 7, n * 126:(n + 1) * 126] = w_int

    va32 = np.zeros((126, 32), np.float32)
    for h in range(H):
        va32[h * D:(h + 1) * D, h] = v_att[h]

    densel = np.zeros((128, 3), np.float32)
    for j in range(4):
        for h in range(H):
            densel[32 * j + h, h] = 1.0

    ebcsel = np.zeros((128, 4 * 126), np.float32)
    for j in range(4):
        for h in range(H):
            ebcsel[32 * j + h, j * 126 + h * D:(j * 126) + (h + 1) * D] = 1.0

    rbc = np.zeros((3, 126), np.float32)
    for h in range(H):
        rbc[h, h * D:(h + 1) * D] = 1.0

    wh1r = np.ascontiguousarray(
        w_h1.reshape(2, 128, HID).transpose(1, 0, 2).reshape(128, 512))
    wh2r = np.ascontiguousarray(
        w_h2.reshape(2, 128, HID).transpose(1, 0, 2).reshape(128, 512))
    woutr = np.ascontiguousarray(
        w_out.reshape(2, 128, NOUT).transpose(1, 0, 2).reshape(128, 8))

    params = {
        "wia": wia.astype(_bf16np), "wib": wib.astype(_bf16np), "wo": w_own.astype(_bf16np),
        "wqb": blockdiag(wq).astype(_bf16np), "wkb": blockdiag(wk).astype(_bf16np), "wvb": blockdiag(wv).astype(_bf16np),
        "va32": va32.astype(_bf16np), "densel": densel.astype(_bf16np), "ebcsel": ebcsel.astype(_bf16np), "rbc": rbc,
        "wat": w_attn, "wop": w_ownp.astype(_bf16np),
        "wh1r": wh1r, "wh2r": wh2r, "woutr": woutr,
        "bown": np.asarray(inputs["b_own"], np.float32).reshape(126, 1),
        "bint": np.asarray(inputs["b_int"], np.float32).reshape(126, 1),
        "bat": np.asarray(inputs["b_attn"], np.float32).reshape(128, 1),
        "bop": np.asarray(inputs["b_ownp"], np.float32).reshape(128, 1),
        "bh1": np.ascontiguousarray(
            np.asarray(inputs["b_h1"], np.float32).reshape(2, 128).T),
        "bh2": np.ascontiguousarray(
            np.asarray(inputs["b_h2"], np.float32).reshape(2, 128).T),
        "bout": np.asarray(inputs["b_out"], np.float32).reshape(4, 1),
    }

    in_maps = []
    for c in range(N_CORES):
        sl = obs[c * BC:(c + 1) * BC]                       # [BC, 147]
        xo = np.ascontiguousarray(sl[:, :OWN_DIM].T).astype(_bf16np)        # [7, BC]
        intr = sl[:, OWN_DIM:].reshape(BC, N_INTR, INT_DIM)  # [BC, 20, 7]
        intrT = intr.transpose(1, 2, 0)                     # [20, 7, BC]
        xa = np.ascontiguousarray(intrT[:18].reshape(126, BC)).astype(_bf16np)
        xb = np.ascontiguousarray(intrT[18:].reshape(14, BC)).astype(_bf16np)
        m = {"xo": xo, "xa": xa, "xb": xb}
        m.update(params)
        in_maps.append(m)
    return in_maps


def _get_nc():
    if "nc" not in _BUILT:
        _BUILT["nc"] = _build_nc()
    return _BUILT["nc"]


def run(inputs, trace=False):
    from concourse.bass_utils import run_bass_kernel_spmd
    nc = _get_nc()
    in_maps = _host_prep(inputs)
    res = run_bass_kernel_spmd(nc, in_maps, core_ids=list(range(N_CORES)),
                               trace=trace)
    outs = [res.results[c]["outT"] for c in range(N_CORES)]   # each [4, BC]
    full = np.concatenate(outs, axis=1).T                     # [B, 4]
    return np.ascontiguousarray(full, dtype=np.float32), res


def kernel(**inputs):
    out, _ = run(inputs, trace=False)
    return out


# revision 40
# speedup vs baseline: 1.2010x; 1.2010x over previous
"""Trainium2 Bass kernel for nn_AttentionSACModel (sparse_attention).

Data-parallel across 8 NeuronCores: obs sharded along batch, params replicated.
On-device layout keeps batch on the matmul free dim (activations stored
feature-major / transposed); all host<->device layout changes happen in numpy.

Design (refactored from the 489us baseline; ~352us traced):
- ctx uses v-linearity: ctx = Wv.T @ (sum_n alpha_n * z_n); the weighted sum
  accumulates through PSUM via per-n Wv matmuls (PE) instead of per-n V tiles
  (kills 160 scalar copies, the gpsimd add tree and the big vector reduces).
- paired activations: z-prelu and energy-tanh read 2 PSUM banks in one
  activation op ([126, 1024]) to amortize the ~400ns fixed access latency.
- softmax denominator: EG groups pre-summed on gpsimd (SBUF bf16), then one
  matmul with the head-selection matrix + vector reciprocal, emitted at the
  E-phase tail so the ctx phase never stalls on it.
- 3-stage stagger inside each tile (z@c, k+q@c-2, scores@c-4) plus ctx/head
  steps of the previous tile interleaved between, split so the PE never sits
  behind a same-iteration scalar dependency; long back-to-back matmul runs
  let the PE p-state ramp toward 2.4 GHz (512-col bf16 matmul ~215ns).
- PSUM budget (8 banks): z-pair 2, kq-pair 2, scores 1, alpha-bcast 1,
  ctx-accum 1, small/head rotating 1.

Notes: fp8 DoubleRow, custom-DVE ops (reciprocal_approx_*) and
partition_broadcast all crash this image's walrus backend — do not use.
"""
import sys

if "/opt/trn_rl_repo" not in sys.path:
    sys.path.insert(0, "/opt/trn_rl_repo")

import numpy as np
import ml_dtypes
_bf16np = ml_dtypes.bfloat16

OWN_DIM = 7
INT_DIM = 7
N_INTR = 20
H = 3
D = 42
TOT = H * D            # 126
ATTN = 128
HID = 256
NOUT = 4
B = 32768
N_CORES = 8
BC = B // N_CORES      # 4096 rows per core
NB = 512               # batch tile (matmul free dim)
NT = BC // NB          # 8 tiles per core
ALPHA = 0.2            # leaky relu slope

_BUILT = {}


def _build_nc():
    import concourse.bacc as bacc
    import concourse.tile as tile
    from concourse import mybir

    f32 = mybir.dt.float32
    f32r = mybir.dt.float32r
    bf16 = mybir.dt.bfloat16
    AF = mybir.ActivationFunctionType
    ALU = mybir.AluOpType

    nc = bacc.Bacc()

    # ---- DRAM I/O ----
    xo_d = nc.dram_tensor("xo", [OWN_DIM, BC], bf16, kind="ExternalInput")
    xa_d = nc.dram_tensor("xa", [126, BC], bf16, kind="ExternalInput")    # interactors 0..17
    xb_d = nc.dram_tensor("xb", [14, BC], bf16, kind="ExternalInput")     # interactors 18,19
    wia_d = nc.dram_tensor("wia", [126, 18 * 126], bf16, kind="ExternalInput")
    wib_d = nc.dram_tensor("wib", [14, 2 * 126], bf16, kind="ExternalInput")
    wo_d = nc.dram_tensor("wo", [7, 126], bf16, kind="ExternalInput")
    wq_d = nc.dram_tensor("wqb", [126, 126], bf16, kind="ExternalInput")
    wk_d = nc.dram_tensor("wkb", [126, 126], bf16, kind="ExternalInput")
    wv_d = nc.dram_tensor("wvb", [126, 126], bf16, kind="ExternalInput")
    va_d = nc.dram_tensor("va32", [126, 32], bf16, kind="ExternalInput")
    ds_d = nc.dram_tensor("densel", [128, 3], bf16, kind="ExternalInput")
    eb_d = nc.dram_tensor("ebcsel", [128, 4 * 126], bf16, kind="ExternalInput")
    rb_d = nc.dram_tensor("rbc", [3, 126], f32r, kind="ExternalInput")
    wat_d = nc.dram_tensor("wat", [126, 128], f32r, kind="ExternalInput")
    wop_d = nc.dram_tensor("wop", [126, 128], bf16, kind="ExternalInput")
    wh1_d = nc.dram_tensor("wh1r", [128, 512], f32r, kind="ExternalInput")
    wh2_d = nc.dram_tensor("wh2r", [128, 512], f32r, kind="ExternalInput")
    wout_d = nc.dram_tensor("woutr", [128, 8], f32r, kind="ExternalInput")
    bown_d = nc.dram_tensor("bown", [126, 1], f32, kind="ExternalInput")
    bint_d = nc.dram_tensor("bint", [126, 1], f32, kind="ExternalInput")
    bat_d = nc.dram_tensor("bat", [128, 1], f32, kind="ExternalInput")
    bop_d = nc.dram_tensor("bop", [128, 1], f32, kind="ExternalInput")
    bh1_d = nc.dram_tensor("bh1", [128, 2], f32, kind="ExternalInput")
    bh2_d = nc.dram_tensor("bh2", [128, 2], f32, kind="ExternalInput")
    bout_d = nc.dram_tensor("bout", [4, 1], f32, kind="ExternalInput")
    out_d = nc.dram_tensor("outT", [NOUT, BC], f32, kind="ExternalOutput")

    with tile.TileContext(nc) as tc:
        with tc.tile_pool(name="const", bufs=1) as cst, \
             tc.tile_pool(name="px", bufs=3) as px, \
             tc.tile_pool(name="pzt", bufs=2) as pzt, \
             tc.tile_pool(name="pen", bufs=4) as pen, \
             tc.tile_pool(name="peg", bufs=2) as peg, \
             tc.tile_pool(name="pn", bufs=8) as pnp, \
             tc.tile_pool(name="ph", bufs=2) as ph, \
             tc.tile_pool(name="pz", bufs=1, space="PSUM") as ppz, \
             tc.tile_pool(name="pk", bufs=1, space="PSUM") as ppk, \
             tc.tile_pool(name="ps", bufs=1, space="PSUM") as pps, \
             tc.tile_pool(name="pb", bufs=1, space="PSUM") as ppb, \
             tc.tile_pool(name="pc", bufs=1, space="PSUM") as ppc, \
             tc.tile_pool(name="sm", bufs=1, space="PSUM") as psm:

            # ---- constants ----
            WiA = cst.tile([126, 18 * 126], bf16)
            WiB = cst.tile([14, 2 * 126], bf16)
            Wo = cst.tile([7, 126], bf16)
            Wq = cst.tile([126, 126], bf16)
            Wk = cst.tile([126, 126], bf16)
            Wv = cst.tile([126, 126], bf16)
            Va = cst.tile([126, 32], bf16)
            Ds = cst.tile([128, 3], bf16)
            Eb = cst.tile([128, 4 * 126], bf16)
            Rb = cst.tile([3, 126], f32r)
            Wat = cst.tile([126, 128], f32r)
            Wop = cst.tile([126, 128], bf16)
            WH1 = cst.tile([128, 512], f32r)
            WH2 = cst.tile([128, 512], f32r)
            WOUT = cst.tile([128, 8], f32r)
            Bown = cst.tile([126, 1], f32)
            Bint = cst.tile([126, 1], f32)
            Bat = cst.tile([128, 1], f32)
            Bop = cst.tile([128, 1], f32)
            BH1 = cst.tile([128, 2], f32)
            BH2 = cst.tile([128, 2], f32)
            Bout = cst.tile([4, 1], f32)
            for t_sb, t_dr in [(WiA, wia_d), (Wo, wo_d), (Bown, bown_d),
                               (Bint, bint_d), (WiB, wib_d), (Wk, wk_d),
                               (Wq, wq_d), (Va, va_d), (Ds, ds_d)]:
                nc.sync.dma_start(out=t_sb, in_=t_dr[:, :])

            def load_late_consts():
                for t_sb, t_dr in [(Eb, eb_d), (Rb, rb_d),
                                   (Wv, wv_d), (Wat, wat_d), (Wop, wop_d),
                                   (WH1, wh1_d), (WH2, wh2_d), (WOUT, wout_d),
                                   (Bat, bat_d), (Bop, bop_d), (BH1, bh1_d),
                                   (BH2, bh2_d), (Bout, bout_d)]:
                    nc.scalar.dma_start(out=t_sb, in_=t_dr[:, :])

            with nc.allow_low_precision(reason="bf16 intermediates; final accums f32"):
                state = {}

                def load_x(t):
                    bs = t * NB
                    XO = px.tile([OWN_DIM, NB], bf16, tag="xo", name="XO")
                    XA = px.tile([126, NB], bf16, tag="xa", name="XA")
                    XB = px.tile([14, NB], bf16, tag="xb", name="XB")
                    nc.sync.dma_start(out=XO, in_=xo_d[:, bs:bs + NB])
                    nc.sync.dma_start(out=XA, in_=xa_d[:, bs:bs + NB])
                    nc.sync.dma_start(out=XB, in_=xb_d[:, bs:bs + NB])
                    state[t] = {"X": (XO, XA, XB)}

                def e_pre(t):
                    """own embed for tile t (sm bank use #1 of the cycle)"""
                    XO = state[t]["X"][0]
                    PO = psm.tile([128, NB], f32, tag="sm", name="PO")
                    nc.tensor.matmul(PO[0:126, :], Wo, XO)
                    OWN = ph.tile([126, NB], bf16, tag="own", name="OWN")
                    nc.scalar.activation(OWN, PO[0:126, :], AF.Prelu,
                                         bias=Bown, alpha=ALPHA)
                    ZT = pzt.tile([126, N_INTR, NB], bf16, tag="zt", name="ZT")
                    EG = peg.tile([128, 5, NB], bf16, tag="eg", name="EG")
                    st = state[t]
                    st.update({"OWN": OWN, "ZT": ZT, "EG": EG})

                def z_mm(t, c):
                    """embed matmuls for chunk c (n=2c, 2c+1) -> pz pair bank"""
                    st = state[t]
                    XA, XB = st["X"][1], st["X"][2]
                    PZ = ppz.tile([126, 2, NB], f32, tag="pz", name="PZ")
                    for i in range(2):
                        n = 2 * c + i
                        if n < 18:
                            nc.tensor.matmul(PZ[:, i, :],
                                             WiA[:, n * 126:(n + 1) * 126], XA)
                        else:
                            nc.tensor.matmul(PZ[:, i, :],
                                             WiB[:, (n - 18) * 126:(n - 17) * 126], XB)
                    st[("PZ", c)] = PZ

                def z_act(t, c):
                    st = state[t]
                    nc.scalar.activation(st["ZT"][:, 2 * c:2 * c + 2, :],
                                         st.pop(("PZ", c)),
                                         AF.Prelu, bias=Bint, alpha=ALPHA)

                def kq_mm(t, c):
                    st = state[t]
                    ZT, OWN = st["ZT"], st["OWN"]
                    PK = ppk.tile([126, 2, NB], f32, tag="pk", name="PK")
                    for i in range(2):
                        n = 2 * c + i
                        nc.tensor.matmul(PK[:, i, :], Wk, ZT[:, n, :],
                                         start=True, stop=False)
                        nc.tensor.matmul(PK[:, i, :], Wq, OWN,
                                         start=False, stop=True)
                    st[("PK", c)] = PK

                def tanh_act(t, c):
                    st = state[t]
                    EN = pen.tile([126, 2, NB], bf16, tag="en", name="EN")
                    nc.scalar.activation(EN, st.pop(("PK", c)), AF.Tanh)
                    st[("EN", c)] = EN

                def score_mm(t, c):
                    st = state[t]
                    EN = st.pop(("EN", c))
                    for i in range(2):
                        n = 2 * c + i
                        j = n % 4
                        if j == 0:
                            st["PS"] = pps.tile([128, NB], f32, tag="ps", name="PS")
                        nc.tensor.matmul(st["PS"][32 * j:32 * (j + 1), :], Va,
                                         EN[:, i, :], tile_position=(0, 32 * j))

                def exp_act(t, g):
                    st = state[t]
                    nc.scalar.activation(st["EG"][:, g, :], st["PS"], AF.Exp)

                def egsum(t):
                    """denominator pre-sum on gpsimd (SBUF bf16 only)"""
                    st = state[t]
                    EG = st["EG"]
                    s01 = ph.tile([128, NB], bf16, tag="es0", name="ES0")
                    s23 = ph.tile([128, NB], bf16, tag="es1", name="ES1")
                    s03 = ph.tile([128, NB], bf16, tag="es2", name="ES2")
                    EGS = ph.tile([128, NB], bf16, tag="egs", name="EGS")
                    nc.gpsimd.tensor_tensor(out=s01, in0=EG[:, 0, :], in1=EG[:, 1, :], op=ALU.add)
                    nc.gpsimd.tensor_tensor(out=s23, in0=EG[:, 2, :], in1=EG[:, 3, :], op=ALU.add)
                    nc.gpsimd.tensor_tensor(out=s03, in0=s01, in1=s23, op=ALU.add)
                    nc.gpsimd.tensor_tensor(out=EGS, in0=s03, in1=EG[:, 4, :], op=ALU.add)
                    st["EGS"] = EGS

                def denom(t):
                    """softmax denominator + reciprocal, emitted at E-phase
                    tail so the ctx phase never stalls on the reciprocal"""
                    st = state[t]
                    PD = psm.tile([128, NB], f32, tag="sm", name="PD")
                    nc.tensor.matmul(PD[0:3, :], Ds, st["EGS"])
                    RD = ph.tile([3, NB], f32r, tag="rd", name="RD")
                    nc.vector.reciprocal(RD, PD[0:3, :])
                    st["RD"] = RD

                def build_c_steps(t):
                    """ctx + head for tile t, as a list of interleavable steps"""
                    st = state[t]
                    bs = t * NB

                    def c_rbcast():
                        PR = psm.tile([128, NB], f32, tag="sm", name="PR")
                        nc.tensor.matmul(PR[0:126, :], Rb, st["RD"])
                        PRs = ph.tile([126, NB], f32, tag="prs", name="PRs")
                        nc.vector.tensor_scalar_mul(PRs, PR[0:126, :], 1.0)
                        st["PR"] = PRs

                    def mk_cna(n):
                        def s():
                            g, j = n // 4, n % 4
                            PEb = ppb.tile([126, NB], f32, tag="pb", name="PEb")
                            nc.tensor.matmul(PEb, Eb[:, j * 126:(j + 1) * 126],
                                             st["EG"][:, g, :])
                            PN = pnp.tile([126, NB], bf16, tag="pn", name="PN")
                            nc.vector.tensor_tensor(out=PN, in0=PEb,
                                                    in1=st["ZT"][:, n, :], op=ALU.mult)
                            st[("PN", n)] = PN
                        return s

                    def mk_cnb(n):
                        def s():
                            if n == 0:
                                st["CTXV"] = ppc.tile([126, NB], f32, tag="pc",
                                                      name="CTXV")
                            nc.tensor.matmul(st["CTXV"], Wv, st.pop(("PN", n)),
                                             start=(n == 0), stop=(n == 19))
                        return s

                    def c_norm():
                        CTX = ph.tile([126, NB], f32r, tag="ctx", name="CTX")
                        nc.vector.tensor_tensor(out=CTX, in0=st["CTXV"],
                                                in1=st["PR"], op=ALU.mult)
                        st["CTX"] = CTX

                    def c_att():
                        PH = psm.tile([128, NB], f32, tag="sm", name="PH")
                        nc.tensor.matmul(PH, Wat, st["CTX"])
                        st["ATT"] = ph.tile([128, NB], f32r, tag="att", name="ATT")
                        nc.scalar.activation(st["ATT"], PH, AF.Tanh, bias=Bat)

                    def c_ownp():
                        PH = psm.tile([128, NB], f32, tag="sm", name="PH2")
                        nc.tensor.matmul(PH, Wop, st["OWN"])
                        st["OWV"] = ph.tile([128, NB], f32r, tag="owv", name="OWV")
                        nc.scalar.activation(st["OWV"], PH, AF.Tanh, bias=Bop)

                    def mk_h1(mh):
                        def s():
                            PHh = psm.tile([128, NB], f32, tag="sm", name="PHh")
                            nc.tensor.matmul(PHh, WH1[:, mh * 128:(mh + 1) * 128],
                                             st["OWV"], start=True, stop=False)
                            nc.tensor.matmul(PHh, WH1[:, 256 + mh * 128:256 + (mh + 1) * 128],
                                             st["ATT"], start=False, stop=True)
                            st[f"H1{mh}"] = ph.tile([128, NB], f32r, tag=f"h1{mh}", name="H1")
                            nc.scalar.activation(st[f"H1{mh}"], PHh, AF.Prelu,
                                                 bias=BH1[:, mh:mh + 1], alpha=ALPHA)
                        return s

                    def mk_h2(mh):
                        def s():
                            PHh = psm.tile([128, NB], f32, tag="sm", name="PHh2")
                            nc.tensor.matmul(PHh, WH2[:, mh * 128:(mh + 1) * 128],
                                             st["H10"], start=True, stop=False)
                            nc.tensor.matmul(PHh, WH2[:, 256 + mh * 128:256 + (mh + 1) * 128],
                                             st["H11"], start=False, stop=True)
                            st[f"H2{mh}"] = ph.tile([128, NB], f32r, tag=f"h2{mh}", name="H2")
                            nc.scalar.activation(st[f"H2{mh}"], PHh, AF.Prelu,
                                                 bias=BH2[:, mh:mh + 1], alpha=ALPHA)
                        return s

                    def c_out():
                        PO4 = psm.tile([128, NB], f32, tag="sm", name="PO4")
                        nc.tensor.matmul(PO4[0:4, :], WOUT[:, 0:4], st["H20"],
                                         start=True, stop=False)
                        nc.tensor.matmul(PO4[0:4, :], WOUT[:, 4:8], st["H21"],
                                         start=False, stop=True)
                        OT = ph.tile([4, NB], f32, tag="ot", name="OT")
                        nc.vector.tensor_scalar_add(OT, PO4[0:4, :], Bout)
                        nc.sync.dma_start(out=out_d[:, bs:bs + NB], in_=OT)
                        del state[t]

                    cns = [c_rbcast, mk_cna(0), mk_cna(1)]
                    for n in range(2, N_INTR):
                        cns += [mk_cnb(n - 2), mk_cna(n)]
                    cns += [mk_cnb(N_INTR - 2), mk_cnb(N_INTR - 1)]
                    return (cns + [c_norm, c_att, c_ownp, mk_h1(0), mk_h1(1),
                                   mk_h2(0), mk_h2(1), c_out])

                def emit_tile(t, csteps):
                    """stage E of tile t interleaved with ctx/head steps of t-1.

                    3-stage stagger inside E: z matmuls of chunk c, k+q of c-1,
                    scores of c-2 — keeps PE fed while scalar drains PSUM."""
                    ci = 0

                    def c_run(k):
                        nonlocal ci
                        for _ in range(k):
                            if ci < len(csteps):
                                csteps[ci]()
                                ci += 1

                    e_pre(t)
                    NCH = N_INTR // 2
                    for c in range(NCH + 4):
                        if c < NCH:
                            z_mm(t, c)
                        c_run(2)
                        if 1 <= c < NCH + 1:
                            z_act(t, c - 1)
                        if 2 <= c < NCH + 2:
                            kq_mm(t, c - 2)
                        c_run(2)
                        if 3 <= c < NCH + 3:
                            tanh_act(t, c - 3)
                        if 4 <= c < NCH + 4:
                            score_mm(t, c - 4)
                            if (c - 4) % 2 == 1:
                                exp_act(t, (c - 4) // 2)
                    egsum(t)
                    denom(t)
                    c_run(len(csteps))

                # ---- software pipeline over tiles ----
                load_x(0)
                load_x(1)
                emit_tile(0, [])
                load_late_consts()
                for t in range(1, NT):
                    if t + 1 < NT:
                        load_x(t + 1)
                    emit_tile(t, build_c_steps(t - 1))
                for s in build_c_steps(NT - 1):
                    s()

    nc.compile()
    return nc


def _host_prep(inputs):
    """Build per-core input maps (numpy only)."""
    obs = np.ascontiguousarray(inputs["obs"], dtype=np.float32)
    w_own = np.asarray(inputs["w_own"], np.float32)
    w_int = np.asarray(inputs["w_int"], np.float32)
    wq = np.asarray(inputs["wq"], np.float32)
    wk = np.asarray(inputs["wk"], np.float32)
    wv = np.asarray(inputs["wv"], np.float32)
    v_att = np.asarray(inputs["v_att"], np.float32)
    w_attn = np.asarray(inputs["w_attn"], np.float32)
    w_ownp = np.asarray(inputs["w_ownp"], np.float32)
    w_h1 = np.asarray(inputs["w_h1"], np.float32)
    w_h2 = np.asarray(inputs["w_h2"], np.float32)
    w_out = np.asarray(inputs["w_out"], np.float32)

    def blockdiag(w):  # [H, D, D] -> [126, 126]
        out = np.zeros((TOT, TOT), np.float32)
        for h in range(H):
            out[h * D:(h + 1) * D, h * D:(h + 1) * D] = w[h]
        return out

    wia = np.zeros((126, 18 * 126), np.float32)
    for n in range(18):
        wia[7 * n:7 * n + 7, n * 126:(n + 1) * 126] = w_int
    wib = np.zeros((14, 2 * 126), np.float32)
    for n in range(2):
        wib[7 * n:7 * n + 7, n * 126:(n + 1) * 126] = w_int

    va32 = np.zeros((126, 32), np.float32)
    for h in range(H):
        va32[h * D:(h + 1) * D, h] = v_att[h]

    densel = np.zeros((128, 3), np.float32)
    for j in range(4):
        for h in range(H):
            densel[32 * j + h, h] = 1.0

    ebcsel = np.zeros((128, 4 * 126), np.float32)
    for j in range(4):
        for h in range(H):
            ebcsel[32 * j + h, j * 126 + h * D:(j * 126) + (h + 1) * D] = 1.0

    rbc = np.zeros((3, 126), np.float32)
    for h in range(H):
        rbc[h, h * D:(h + 1) * D] = 1.0

    wh1r = np.ascontiguousarray(
        w_h1.reshape(2, 128, HID).transpose(1, 0, 2).reshape(128, 512))
    wh2r = np.ascontiguousarray(
        w_h2.reshape(2, 128, HID).transpose(1, 0, 2).reshape(128, 512))
    woutr = np.ascontiguousarray(
        w_out.reshape(2, 128, NOUT).transpose(1, 0, 2).reshape(128, 8))

    params = {
        "wia": wia.astype(_bf16np), "wib": wib.astype(_bf16np), "wo": w_own.astype(_bf16np),
        "wqb": blockdiag(wq).astype(_bf16np), "wkb": blockdiag(wk).astype(_bf16np), "wvb": blockdiag(wv).astype(_bf16np),
        "va32": va32.astype(_bf16np), "densel": densel.astype(_bf16np), "ebcsel": ebcsel.astype(_bf16np), "rbc": rbc,
        "wat": w_attn, "wop": w_ownp.astype(_bf16np),
        "wh1r": wh1r, "wh2r": wh2r, "woutr": woutr,
        "bown": np.asarray(inputs["b_own"], np.float32).reshape(126, 1),
        "bint": np.asarray(inputs["b_int"], np.float32).reshape(126, 1),
        "bat": np.asarray(inputs["b_attn"], np.float32).reshape(128, 1),
        "bop": np.asarray(inputs["b_ownp"], np.float32).reshape(128, 1),
        "bh1": np.ascontiguousarray(
            np.asarray(inputs["b_h1"], np.float32).reshape(2, 128).T),
        "bh2": np.ascontiguousarray(
            np.asarray(inputs["b_h2"], np.float32).reshape(2, 128).T),
        "bout": np.asarray(inputs["b_out"], np.float32).reshape(4, 1),
    }

    in_maps = []
    for c in range(N_CORES):
        sl = obs[c * BC:(c + 1) * BC]                       # [BC, 147]
        xo = np.ascontiguousarray(sl[:, :OWN_DIM].T).astype(_bf16np)        # [7, BC]
        intr = sl[:, OWN_DIM:].reshape(BC, N_INTR, INT_DIM)  # [BC, 20, 7]
        intrT = intr.transpose(1, 2, 0)                     # [20, 7, BC]
        xa = np.ascontiguousarray(intrT[:18].reshape(126, BC)).astype(_bf16np)
        xb = np.ascontiguousarray(intrT[18:].reshape(14, BC)).astype(_bf16np)
        m = {"xo": xo, "xa": xa, "xb": xb}
        m.update(params)
        in_maps.append(m)
    return in_maps


def _get_nc():
    if "nc" not in _BUILT:
        _BUILT["nc"] = _build_nc()
    return _BUILT["nc"]


def run(inputs, trace=False):
    from concourse.bass_utils import run_bass_kernel_spmd
    nc = _get_nc()
    in_maps = _host_prep(inputs)
    res = run_bass_kernel_spmd(nc, in_maps, core_ids=list(range(N_CORES)),
                               trace=trace)
    outs = [res.results[c]["outT"] for c in range(N_CORES)]   # each [4, BC]
    full = np.concatenate(outs, axis=1).T                     # [B, 4]
    return np.ascontiguousarray(full, dtype=np.float32), res


def kernel(**inputs):
    out, _ = run(inputs, trace=False)
    return out


# revision 41
# speedup vs baseline: 1.2028x; 1.0015x over previous
"""Trainium2 Bass kernel for nn_AttentionSACModel (sparse_attention).

Data-parallel across 8 NeuronCores: obs sharded along batch, params replicated.
On-device layout keeps batch on the matmul free dim (activations stored
feature-major / transposed); all host<->device layout changes happen in numpy.

Design (refactored from the 489us baseline; ~352us traced):
- ctx uses v-linearity: ctx = Wv.T @ (sum_n alpha_n * z_n); the weighted sum
  accumulates through PSUM via per-n Wv matmuls (PE) instead of per-n V tiles
  (kills 160 scalar copies, the gpsimd add tree and the big vector reduces).
- paired activations: z-prelu and energy-tanh read 2 PSUM banks in one
  activation op ([126, 1024]) to amortize the ~400ns fixed access latency.
- softmax denominator: EG groups pre-summed on gpsimd (SBUF bf16), then one
  matmul with the head-selection matrix + vector reciprocal, emitted at the
  E-phase tail so the ctx phase never stalls on it.
- 3-stage stagger inside each tile (z@c, k+q@c-2, scores@c-4) plus ctx/head
  steps of the previous tile interleaved between, split so the PE never sits
  behind a same-iteration scalar dependency; long back-to-back matmul runs
  let the PE p-state ramp toward 2.4 GHz (512-col bf16 matmul ~215ns).
- PSUM budget (8 banks): z-pair 2, kq-pair 2, scores 1, alpha-bcast 1,
  ctx-accum 1, small/head rotating 1.

Notes: fp8 DoubleRow, custom-DVE ops (reciprocal_approx_*) and
partition_broadcast all crash this image's walrus backend — do not use.
"""
import sys

if "/opt/trn_rl_repo" not in sys.path:
    sys.path.insert(0, "/opt/trn_rl_repo")

import numpy as np
import ml_dtypes
_bf16np = ml_dtypes.bfloat16

OWN_DIM = 7
INT_DIM = 7
N_INTR = 20
H = 3
D = 42
TOT = H * D            # 126
ATTN = 128
HID = 256
NOUT = 4
B = 32768
N_CORES = 8
BC = B // N_CORES      # 4096 rows per core
NB = 512               # batch tile (matmul free dim)
NT = BC // NB          # 8 tiles per core
ALPHA = 0.2            # leaky relu slope

_BUILT = {}


def _build_nc():
    import concourse.bacc as bacc
    import concourse.tile as tile
    from concourse import mybir

    f32 = mybir.dt.float32
    f32r = mybir.dt.float32r
    bf16 = mybir.dt.bfloat16
    AF = mybir.ActivationFunctionType
    ALU = mybir.AluOpType

    nc = bacc.Bacc()

    # ---- DRAM I/O ----
    xo_d = nc.dram_tensor("xo", [OWN_DIM, BC], bf16, kind="ExternalInput")
    xa_d = nc.dram_tensor("xa", [126, BC], bf16, kind="ExternalInput")    # interactors 0..17
    xb_d = nc.dram_tensor("xb", [14, BC], bf16, kind="ExternalInput")     # interactors 18,19
    wia_d = nc.dram_tensor("wia", [126, 18 * 126], bf16, kind="ExternalInput")
    wib_d = nc.dram_tensor("wib", [14, 2 * 126], bf16, kind="ExternalInput")
    wo_d = nc.dram_tensor("wo", [7, 126], bf16, kind="ExternalInput")
    wq_d = nc.dram_tensor("wqb", [126, 126], bf16, kind="ExternalInput")
    wk_d = nc.dram_tensor("wkb", [126, 126], bf16, kind="ExternalInput")
    wv_d = nc.dram_tensor("wvb", [126, 126], bf16, kind="ExternalInput")
    va_d = nc.dram_tensor("va32", [126, 32], bf16, kind="ExternalInput")
    ds_d = nc.dram_tensor("densel", [128, 3], bf16, kind="ExternalInput")
    eb_d = nc.dram_tensor("ebcsel", [128, 4 * 126], bf16, kind="ExternalInput")
    rb_d = nc.dram_tensor("rbc", [3, 126], f32r, kind="ExternalInput")
    wat_d = nc.dram_tensor("wat", [126, 128], f32r, kind="ExternalInput")
    wop_d = nc.dram_tensor("wop", [126, 128], bf16, kind="ExternalInput")
    wh1_d = nc.dram_tensor("wh1r", [128, 512], f32r, kind="ExternalInput")
    wh2_d = nc.dram_tensor("wh2r", [128, 512], f32r, kind="ExternalInput")
    wout_d = nc.dram_tensor("woutr", [128, 8], f32r, kind="ExternalInput")
    bown_d = nc.dram_tensor("bown", [126, 1], f32, kind="ExternalInput")
    bint_d = nc.dram_tensor("bint", [126, 1], f32, kind="ExternalInput")
    bat_d = nc.dram_tensor("bat", [128, 1], f32, kind="ExternalInput")
    bop_d = nc.dram_tensor("bop", [128, 1], f32, kind="ExternalInput")
    bh1_d = nc.dram_tensor("bh1", [128, 2], f32, kind="ExternalInput")
    bh2_d = nc.dram_tensor("bh2", [128, 2], f32, kind="ExternalInput")
    bout_d = nc.dram_tensor("bout", [4, 1], f32, kind="ExternalInput")
    out_d = nc.dram_tensor("outT", [NOUT, BC], f32, kind="ExternalOutput")

    with tile.TileContext(nc) as tc:
        with tc.tile_pool(name="const", bufs=1) as cst, \
             tc.tile_pool(name="px", bufs=3) as px, \
             tc.tile_pool(name="pzt", bufs=2) as pzt, \
             tc.tile_pool(name="pen", bufs=4) as pen, \
             tc.tile_pool(name="peg", bufs=2) as peg, \
             tc.tile_pool(name="pn", bufs=8) as pnp, \
             tc.tile_pool(name="ph", bufs=2) as ph, \
             tc.tile_pool(name="pz", bufs=1, space="PSUM") as ppz, \
             tc.tile_pool(name="pk", bufs=1, space="PSUM") as ppk, \
             tc.tile_pool(name="ps", bufs=1, space="PSUM") as pps, \
             tc.tile_pool(name="pb", bufs=1, space="PSUM") as ppb, \
             tc.tile_pool(name="pc", bufs=1, space="PSUM") as ppc, \
             tc.tile_pool(name="sm", bufs=1, space="PSUM") as psm:

            # ---- constants ----
            WiA = cst.tile([126, 18 * 126], bf16)
            WiB = cst.tile([14, 2 * 126], bf16)
            Wo = cst.tile([7, 126], bf16)
            Wq = cst.tile([126, 126], bf16)
            Wk = cst.tile([126, 126], bf16)
            Wv = cst.tile([126, 126], bf16)
            Va = cst.tile([126, 32], bf16)
            Ds = cst.tile([128, 3], bf16)
            Eb = cst.tile([128, 4 * 126], bf16)
            Rb = cst.tile([3, 126], f32r)
            Wat = cst.tile([126, 128], f32r)
            Wop = cst.tile([126, 128], bf16)
            WH1 = cst.tile([128, 512], f32r)
            WH2 = cst.tile([128, 512], f32r)
            WOUT = cst.tile([128, 8], f32r)
            Bown = cst.tile([126, 1], f32)
            Bint = cst.tile([126, 1], f32)
            Bat = cst.tile([128, 1], f32)
            Bop = cst.tile([128, 1], f32)
            BH1 = cst.tile([128, 2], f32)
            BH2 = cst.tile([128, 2], f32)
            Bout = cst.tile([4, 1], f32)
            for t_sb, t_dr in [(WiA, wia_d), (Wo, wo_d), (Bown, bown_d),
                               (Bint, bint_d), (WiB, wib_d), (Wk, wk_d),
                               (Wq, wq_d), (Va, va_d), (Ds, ds_d)]:
                nc.sync.dma_start(out=t_sb, in_=t_dr[:, :])

            def load_late_consts():
                for t_sb, t_dr in [(Eb, eb_d), (Rb, rb_d),
                                   (Wv, wv_d), (Wat, wat_d), (Wop, wop_d),
                                   (WH1, wh1_d), (WH2, wh2_d), (WOUT, wout_d),
                                   (Bat, bat_d), (Bop, bop_d), (BH1, bh1_d),
                                   (BH2, bh2_d), (Bout, bout_d)]:
                    nc.scalar.dma_start(out=t_sb, in_=t_dr[:, :])

            with nc.allow_low_precision(reason="bf16 intermediates; final accums f32"):
                state = {}

                def load_x(t):
                    bs = t * NB
                    XO = px.tile([OWN_DIM, NB], bf16, tag="xo", name="XO")
                    XA = px.tile([126, NB], bf16, tag="xa", name="XA")
                    XB = px.tile([14, NB], bf16, tag="xb", name="XB")
                    nc.sync.dma_start(out=XO, in_=xo_d[:, bs:bs + NB])
                    nc.sync.dma_start(out=XA, in_=xa_d[:, bs:bs + NB])
                    nc.sync.dma_start(out=XB, in_=xb_d[:, bs:bs + NB])
                    state[t] = {"X": (XO, XA, XB)}

                def e_pre(t):
                    """own embed for tile t (sm bank use #1 of the cycle)"""
                    XO = state[t]["X"][0]
                    PO = psm.tile([128, NB], f32, tag="sm", name="PO")
                    nc.tensor.matmul(PO[0:126, :], Wo, XO)
                    OWN = ph.tile([126, NB], bf16, tag="own", name="OWN")
                    nc.scalar.activation(OWN, PO[0:126, :], AF.Prelu,
                                         bias=Bown, alpha=ALPHA)
                    ZT = pzt.tile([126, N_INTR, NB], bf16, tag="zt", name="ZT")
                    EG = peg.tile([128, 5, NB], bf16, tag="eg", name="EG")
                    st = state[t]
                    st.update({"OWN": OWN, "ZT": ZT, "EG": EG})

                def z_mm(t, c):
                    """embed matmuls for chunk c (n=2c, 2c+1) -> pz pair bank"""
                    st = state[t]
                    XA, XB = st["X"][1], st["X"][2]
                    PZ = ppz.tile([126, 2, NB], f32, tag="pz", name="PZ")
                    for i in range(2):
                        n = 2 * c + i
                        if n < 18:
                            nc.tensor.matmul(PZ[:, i, :],
                                             WiA[:, n * 126:(n + 1) * 126], XA)
                        else:
                            nc.tensor.matmul(PZ[:, i, :],
                                             WiB[:, (n - 18) * 126:(n - 17) * 126], XB)
                    st[("PZ", c)] = PZ

                def z_act(t, c):
                    st = state[t]
                    nc.scalar.activation(st["ZT"][:, 2 * c:2 * c + 2, :],
                                         st.pop(("PZ", c)),
                                         AF.Prelu, bias=Bint, alpha=ALPHA)

                def kq_mm(t, c):
                    st = state[t]
                    ZT, OWN = st["ZT"], st["OWN"]
                    PK = ppk.tile([126, 2, NB], f32, tag="pk", name="PK")
                    for i in range(2):
                        n = 2 * c + i
                        nc.tensor.matmul(PK[:, i, :], Wk, ZT[:, n, :],
                                         start=True, stop=False)
                        nc.tensor.matmul(PK[:, i, :], Wq, OWN,
                                         start=False, stop=True)
                    st[("PK", c)] = PK

                def tanh_act(t, c):
                    st = state[t]
                    EN = pen.tile([126, 2, NB], bf16, tag="en", name="EN")
                    nc.scalar.activation(EN, st.pop(("PK", c)), AF.Tanh)
                    st[("EN", c)] = EN

                def score_mm(t, c):
                    st = state[t]
                    EN = st.pop(("EN", c))
                    for i in range(2):
                        n = 2 * c + i
                        j = n % 4
                        if j == 0:
                            st["PS"] = pps.tile([128, NB], f32, tag="ps", name="PS")
                        nc.tensor.matmul(st["PS"][32 * j:32 * (j + 1), :], Va,
                                         EN[:, i, :], tile_position=(0, 32 * j))

                def exp_act(t, g):
                    st = state[t]
                    nc.scalar.activation(st["EG"][:, g, :], st["PS"], AF.Exp)

                def egsum(t):
                    """denominator pre-sum on gpsimd (SBUF bf16 only)"""
                    st = state[t]
                    EG = st["EG"]
                    s01 = ph.tile([128, NB], bf16, tag="es0", name="ES0")
                    s23 = ph.tile([128, NB], bf16, tag="es1", name="ES1")
                    s03 = ph.tile([128, NB], bf16, tag="es2", name="ES2")
                    EGS = ph.tile([128, NB], bf16, tag="egs", name="EGS")
                    nc.gpsimd.tensor_tensor(out=s01, in0=EG[:, 0, :], in1=EG[:, 1, :], op=ALU.add)
                    nc.gpsimd.tensor_tensor(out=s23, in0=EG[:, 2, :], in1=EG[:, 3, :], op=ALU.add)
                    nc.gpsimd.tensor_tensor(out=s03, in0=s01, in1=s23, op=ALU.add)
                    nc.gpsimd.tensor_tensor(out=EGS, in0=s03, in1=EG[:, 4, :], op=ALU.add)
                    st["EGS"] = EGS

                def denom(t):
                    """softmax denominator + reciprocal, emitted at E-phase
                    tail so the ctx phase never stalls on the reciprocal"""
                    st = state[t]
                    PD = psm.tile([128, NB], f32, tag="sm", name="PD")
                    nc.tensor.matmul(PD[0:3, :], Ds, st["EGS"])
                    RD = ph.tile([3, NB], f32r, tag="rd", name="RD")
                    nc.vector.reciprocal(RD, PD[0:3, :])
                    st["RD"] = RD

                def build_c_steps(t):
                    """ctx + head for tile t, as a list of interleavable steps"""
                    st = state[t]
                    bs = t * NB

                    def c_rbcast():
                        PR = psm.tile([128, NB], f32, tag="sm", name="PR")
                        nc.tensor.matmul(PR[0:126, :], Rb, st["RD"])
                        PRs = ph.tile([126, NB], f32, tag="prs", name="PRs")
                        nc.vector.tensor_scalar_mul(PRs, PR[0:126, :], 1.0)
                        st["PR"] = PRs

                    def mk_cna(n):
                        def s():
                            g, j = n // 4, n % 4
                            PEb = ppb.tile([126, NB], f32, tag="pb", name="PEb")
                            nc.tensor.matmul(PEb, Eb[:, j * 126:(j + 1) * 126],
                                             st["EG"][:, g, :])
                            PN = pnp.tile([126, NB], bf16, tag="pn", name="PN")
                            nc.vector.tensor_tensor(out=PN, in0=PEb,
                                                    in1=st["ZT"][:, n, :], op=ALU.mult)
                            st[("PN", n)] = PN
                        return s

                    def mk_cnb(n):
                        def s():
                            if n == 0:
                                st["CTXV"] = ppc.tile([126, NB], f32, tag="pc",
                                                      name="CTXV")
                            nc.tensor.matmul(st["CTXV"], Wv, st.pop(("PN", n)),
                                             start=(n == 0), stop=(n == 19))
                        return s

                    def c_norm():
                        CTX = ph.tile([126, NB], f32r, tag="ctx", name="CTX")
                        nc.vector.tensor_tensor(out=CTX, in0=st["CTXV"],
                                                in1=st["PR"], op=ALU.mult)
                        st["CTX"] = CTX

                    def c_att():
                        PH = psm.tile([128, NB], f32, tag="sm", name="PH")
                        nc.tensor.matmul(PH, Wat, st["CTX"])
                        st["ATT"] = ph.tile([128, NB], f32r, tag="att", name="ATT")
                        nc.scalar.activation(st["ATT"], PH, AF.Tanh, bias=Bat)

                    def c_ownp():
                        PH = psm.tile([128, NB], f32, tag="sm", name="PH2")
                        nc.tensor.matmul(PH, Wop, st["OWN"])
                        st["OWV"] = ph.tile([128, NB], f32r, tag="owv", name="OWV")
                        nc.scalar.activation(st["OWV"], PH, AF.Tanh, bias=Bop)

                    def mk_h1(mh):
                        def s():
                            PHh = psm.tile([128, NB], f32, tag="sm", name="PHh")
                            nc.tensor.matmul(PHh, WH1[:, mh * 128:(mh + 1) * 128],
                                             st["OWV"], start=True, stop=False)
                            nc.tensor.matmul(PHh, WH1[:, 256 + mh * 128:256 + (mh + 1) * 128],
                                             st["ATT"], start=False, stop=True)
                            st[f"H1{mh}"] = ph.tile([128, NB], f32r, tag=f"h1{mh}", name="H1")
                            nc.scalar.activation(st[f"H1{mh}"], PHh, AF.Prelu,
                                                 bias=BH1[:, mh:mh + 1], alpha=ALPHA)
                        return s

                    def mk_h2(mh):
                        def s():
                            PHh = psm.tile([128, NB], f32, tag="sm", name="PHh2")
                            nc.tensor.matmul(PHh, WH2[:, mh * 128:(mh + 1) * 128],
                                             st["H10"], start=True, stop=False)
                            nc.tensor.matmul(PHh, WH2[:, 256 + mh * 128:256 + (mh + 1) * 128],
                                             st["H11"], start=False, stop=True)
                            st[f"H2{mh}"] = ph.tile([128, NB], f32r, tag=f"h2{mh}", name="H2")
                            nc.scalar.activation(st[f"H2{mh}"], PHh, AF.Prelu,
                                                 bias=BH2[:, mh:mh + 1], alpha=ALPHA)
                        return s

                    def c_out():
                        PO4 = psm.tile([128, NB], f32, tag="sm", name="PO4")
                        nc.tensor.matmul(PO4[0:4, :], WOUT[:, 0:4], st["H20"],
                                         start=True, stop=False)
                        nc.tensor.matmul(PO4[0:4, :], WOUT[:, 4:8], st["H21"],
                                         start=False, stop=True)
                        OT = ph.tile([4, NB], f32, tag="ot", name="OT")
                        nc.vector.tensor_scalar_add(OT, PO4[0:4, :], Bout)
                        nc.sync.dma_start(out=out_d[:, bs:bs + NB], in_=OT)
                        del state[t]

                    cns = [c_rbcast, mk_cna(0), mk_cna(1)]
                    for n in range(2, N_INTR):
                        cns += [mk_cnb(n - 2), mk_cna(n)]
                    cns += [mk_cnb(N_INTR - 2), mk_cnb(N_INTR - 1)]
                    return (cns + [c_norm, c_att, c_ownp, mk_h1(0), mk_h1(1),
                                   mk_h2(0), mk_h2(1), c_out])

                def emit_tile(t, csteps):
                    """stage E of tile t interleaved with ctx/head steps of t-1.

                    3-stage stagger inside E: z matmuls of chunk c, k+q of c-1,
                    scores of c-2 — keeps PE fed while scalar drains PSUM."""
                    ci = 0

                    def c_run(k):
                        nonlocal ci
                        for _ in range(k):
                            if ci < len(csteps):
                                csteps[ci]()
                                ci += 1

                    e_pre(t)
                    NCH = N_INTR // 2
                    for c in range(NCH + 5):
                        if c < NCH:
                            z_mm(t, c)
                        c_run(2)
                        if 1 <= c < NCH + 1:
                            z_act(t, c - 1)
                        if 3 <= c < NCH + 3:
                            kq_mm(t, c - 3)
                        c_run(2)
                        if 4 <= c < NCH + 4:
                            tanh_act(t, c - 4)
                        if 5 <= c < NCH + 5:
                            score_mm(t, c - 5)
                            if (c - 5) % 2 == 1:
                                exp_act(t, (c - 5) // 2)
                    egsum(t)
                    denom(t)
                    c_run(len(csteps))

                # ---- software pipeline over tiles ----
                load_x(0)
                load_x(1)
                emit_tile(0, [])
                load_late_consts()
                for t in range(1, NT):
                    if t + 1 < NT:
                        load_x(t + 1)
                    emit_tile(t, build_c_steps(t - 1))
                for s in build_c_steps(NT - 1):
                    s()

    nc.compile()
    return nc


def _host_prep(inputs):
    """Build per-core input maps (numpy only)."""
    obs = np.ascontiguousarray(inputs["obs"], dtype=np.float32)
    w_own = np.asarray(inputs["w_own"], np.float32)
    w_int = np.asarray(inputs["w_int"], np.float32)
    wq = np.asarray(inputs["wq"], np.float32)
    wk = np.asarray(inputs["wk"], np.float32)
    wv = np.asarray(inputs["wv"], np.float32)
    v_att = np.asarray(inputs["v_att"], np.float32)
    w_attn = np.asarray(inputs["w_attn"], np.float32)
    w_ownp = np.asarray(inputs["w_ownp"], np.float32)
    w_h1 = np.asarray(inputs["w_h1"], np.float32)
    w_h2 = np.asarray(inputs["w_h2"], np.float32)
    w_out = np.asarray(inputs["w_out"], np.float32)

    def blockdiag(w):  # [H, D, D] -> [126, 126]
        out = np.zeros((TOT, TOT), np.float32)
        for h in range(H):
            out[h * D:(h + 1) * D, h * D:(h + 1) * D] = w[h]
        return out

    wia = np.zeros((126, 18 * 126), np.float32)
    for n in range(18):
        wia[7 * n:7 * n + 7, n * 126:(n + 1) * 126] = w_int
    wib = np.zeros((14, 2 * 126), np.float32)
    for n in range(2):
        wib[7 * n:7 * n + 7, n * 126:(n + 1) * 126] = w_int

    va32 = np.zeros((126, 32), np.float32)
    for h in range(H):
        va32[h * D:(h + 1) * D, h] = v_att[h]

    densel = np.zeros((128, 3), np.float32)
    for j in range(4):
        for h in range(H):
            densel[32 * j + h, h] = 1.0

    ebcsel = np.zeros((128, 4 * 126), np.float32)
    for j in range(4):
        for h in range(H):
            ebcsel[32 * j + h, j * 126 + h * D:(j * 126) + (h + 1) * D] = 1.0

    rbc = np.zeros((3, 126), np.float32)
    for h in range(H):
        rbc[h, h * D:(h + 1) * D] = 1.0

    wh1r = np.ascontiguousarray(
        w_h1.reshape(2, 128, HID).transpose(1, 0, 2).reshape(128, 512))
    wh2r = np.ascontiguousarray(
        w_h2.reshape(2, 128, HID).transpose(1, 0, 2).reshape(128, 512))
    woutr = np.ascontiguousarray(
        w_out.reshape(2, 128, NOUT).transpose(1, 0, 2).reshape(128, 8))

    params = {
        "wia": wia.astype(_bf16np), "wib": wib.astype(_bf16np), "wo": w_own.astype(_bf16np),
        "wqb": blockdiag(wq).astype(_bf16np), "wkb": blockdiag(wk).astype(_bf16np), "wvb": blockdiag(wv).astype(_bf16np),
        "va32": va32.astype(_bf16np), "densel": densel.astype(_bf16np), "ebcsel": ebcsel.astype(_bf16np), "rbc": rbc,
        "wat": w_attn, "wop": w_ownp.astype(_bf16np),
        "wh1r": wh1r, "wh2r": wh2r, "woutr": woutr,
        "bown": np.asarray(inputs["b_own"], np.float32).reshape(126, 1),
        "bint": np.asarray(inputs["b_int"], np.float32).reshape(126, 1),
        "bat": np.asarray(inputs["b_attn"], np.float32).reshape(128, 1),
        "bop": np.asarray(inputs["b_ownp"], np.float32).reshape(128, 1),
        "bh1": np.ascontiguousarray(
            np.asarray(inputs["b_h1"], np.float32).reshape(2, 128).T),
        "bh2": np.ascontiguousarray(
            np.asarray(inputs["b_h2"], np.float32).reshape(2, 128).T),
        "bout": np.asarray(inputs["b_out"], np.float32).reshape(4, 1),
    }

    in_maps = []
    for c in range(N_CORES):
        sl = obs[c * BC:(c + 1) * BC]                       # [BC, 147]
        xo = np.ascontiguousarray(sl[:, :OWN_DIM].T).astype(_bf16np)        # [7, BC]
        intr = sl[:, OWN_DIM:].reshape(BC, N_INTR, INT_DIM)  # [BC, 20, 7]
        intrT = intr.transpose(1, 2, 0)                     # [20, 7, BC]
        xa = np.ascontiguousarray(intrT[:18].reshape(126, BC)).astype(_bf16np)
        xb = np.ascontiguousarray(intrT[18:].reshape(14, BC)).astype(_bf16np)
        m = {"xo": xo, "xa": xa, "xb": xb}
        m.update(params)
        in_maps.append(m)
    return in_maps


def _get_nc():
    if "nc" not in _BUILT:
        _BUILT["nc"] = _build_nc()
    return _BUILT["nc"]


def run(inputs, trace=False):
    from concourse.bass_utils import run_bass_kernel_spmd
    nc = _get_nc()
    in_maps = _host_prep(inputs)
    res = run_bass_kernel_spmd(nc, in_maps, core_ids=list(range(N_CORES)),
                               trace=trace)
    outs = [res.results[c]["outT"] for c in range(N_CORES)]   # each [4, BC]
    full = np.concatenate(outs, axis=1).T                     # [B, 4]
    return np.ascontiguousarray(full, dtype=np.float32), res


def kernel(**inputs):
    out, _ = run(inputs, trace=False)
    return out


# revision 42
# speedup vs baseline: 1.2031x; 1.0003x over previous
"""Trainium2 Bass kernel for nn_AttentionSACModel (sparse_attention).

Data-parallel across 8 NeuronCores: obs sharded along batch, params replicated.
On-device layout keeps batch on the matmul free dim (activations stored
feature-major / transposed); all host<->device layout changes happen in numpy.

Design (refactored from the 489us baseline; ~352us traced):
- ctx uses v-linearity: ctx = Wv.T @ (sum_n alpha_n * z_n); the weighted sum
  accumulates through PSUM via per-n Wv matmuls (PE) instead of per-n V tiles
  (kills 160 scalar copies, the gpsimd add tree and the big vector reduces).
- paired activations: z-prelu and energy-tanh read 2 PSUM banks in one
  activation op ([126, 1024]) to amortize the ~400ns fixed access latency.
- softmax denominator: EG groups pre-summed on gpsimd (SBUF bf16), then one
  matmul with the head-selection matrix + vector reciprocal, emitted at the
  E-phase tail so the ctx phase never stalls on it.
- 3-stage stagger inside each tile (z@c, k+q@c-2, scores@c-4) plus ctx/head
  steps of the previous tile interleaved between, split so the PE never sits
  behind a same-iteration scalar dependency; long back-to-back matmul runs
  let the PE p-state ramp toward 2.4 GHz (512-col bf16 matmul ~215ns).
- PSUM budget (8 banks): z-pair 2, kq-pair 2, scores 1, alpha-bcast 1,
  ctx-accum 1, small/head rotating 1.

Notes: fp8 DoubleRow, custom-DVE ops (reciprocal_approx_*) and
partition_broadcast all crash this image's walrus backend — do not use.
"""
import sys

if "/opt/trn_rl_repo" not in sys.path:
    sys.path.insert(0, "/opt/trn_rl_repo")

import numpy as np
import ml_dtypes
_bf16np = ml_dtypes.bfloat16

OWN_DIM = 7
INT_DIM = 7
N_INTR = 20
H = 3
D = 42
TOT = H * D            # 126
ATTN = 128
HID = 256
NOUT = 4
B = 32768
N_CORES = 8
BC = B // N_CORES      # 4096 rows per core
NB = 512               # batch tile (matmul free dim)
NT = BC // NB          # 8 tiles per core
ALPHA = 0.2            # leaky relu slope

_BUILT = {}


def _build_nc():
    import concourse.bacc as bacc
    import concourse.tile as tile
    from concourse import mybir

    f32 = mybir.dt.float32
    f32r = mybir.dt.float32r
    bf16 = mybir.dt.bfloat16
    AF = mybir.ActivationFunctionType
    ALU = mybir.AluOpType

    nc = bacc.Bacc()

    # ---- DRAM I/O ----
    xo_d = nc.dram_tensor("xo", [OWN_DIM, BC], bf16, kind="ExternalInput")
    xa_d = nc.dram_tensor("xa", [126, BC], bf16, kind="ExternalInput")    # interactors 0..17
    xb_d = nc.dram_tensor("xb", [14, BC], bf16, kind="ExternalInput")     # interactors 18,19
    wia_d = nc.dram_tensor("wia", [126, 18 * 126], bf16, kind="ExternalInput")
    wib_d = nc.dram_tensor("wib", [14, 2 * 126], bf16, kind="ExternalInput")
    wo_d = nc.dram_tensor("wo", [7, 126], bf16, kind="ExternalInput")
    wq_d = nc.dram_tensor("wqb", [126, 126], bf16, kind="ExternalInput")
    wk_d = nc.dram_tensor("wkb", [126, 126], bf16, kind="ExternalInput")
    wv_d = nc.dram_tensor("wvb", [126, 126], bf16, kind="ExternalInput")
    va_d = nc.dram_tensor("va32", [126, 32], bf16, kind="ExternalInput")
    ds_d = nc.dram_tensor("densel", [128, 3], bf16, kind="ExternalInput")
    eb_d = nc.dram_tensor("ebcsel", [128, 4 * 126], bf16, kind="ExternalInput")
    rb_d = nc.dram_tensor("rbc", [3, 126], f32r, kind="ExternalInput")
    wat_d = nc.dram_tensor("wat", [126, 128], f32r, kind="ExternalInput")
    wop_d = nc.dram_tensor("wop", [126, 128], bf16, kind="ExternalInput")
    wh1_d = nc.dram_tensor("wh1r", [128, 512], f32r, kind="ExternalInput")
    wh2_d = nc.dram_tensor("wh2r", [128, 512], f32r, kind="ExternalInput")
    wout_d = nc.dram_tensor("woutr", [128, 8], f32r, kind="ExternalInput")
    bown_d = nc.dram_tensor("bown", [126, 1], f32, kind="ExternalInput")
    bint_d = nc.dram_tensor("bint", [126, 1], f32, kind="ExternalInput")
    bat_d = nc.dram_tensor("bat", [128, 1], f32, kind="ExternalInput")
    bop_d = nc.dram_tensor("bop", [128, 1], f32, kind="ExternalInput")
    bh1_d = nc.dram_tensor("bh1", [128, 2], f32, kind="ExternalInput")
    bh2_d = nc.dram_tensor("bh2", [128, 2], f32, kind="ExternalInput")
    bout_d = nc.dram_tensor("bout", [4, 1], f32, kind="ExternalInput")
    out_d = nc.dram_tensor("outT", [NOUT, BC], f32, kind="ExternalOutput")

    with tile.TileContext(nc) as tc:
        with tc.tile_pool(name="const", bufs=1) as cst, \
             tc.tile_pool(name="px", bufs=3) as px, \
             tc.tile_pool(name="pzt", bufs=2) as pzt, \
             tc.tile_pool(name="pen", bufs=4) as pen, \
             tc.tile_pool(name="peg", bufs=2) as peg, \
             tc.tile_pool(name="pn", bufs=8) as pnp, \
             tc.tile_pool(name="ph", bufs=2) as ph, \
             tc.tile_pool(name="pz", bufs=1, space="PSUM") as ppz, \
             tc.tile_pool(name="pk", bufs=1, space="PSUM") as ppk, \
             tc.tile_pool(name="ps", bufs=1, space="PSUM") as pps, \
             tc.tile_pool(name="pb", bufs=1, space="PSUM") as ppb, \
             tc.tile_pool(name="pc", bufs=1, space="PSUM") as ppc, \
             tc.tile_pool(name="sm", bufs=1, space="PSUM") as psm:

            # ---- constants ----
            WiA = cst.tile([126, 18 * 126], bf16)
            WiB = cst.tile([14, 2 * 126], bf16)
            Wo = cst.tile([7, 126], bf16)
            Wq = cst.tile([126, 126], bf16)
            Wk = cst.tile([126, 126], bf16)
            Wv = cst.tile([126, 126], bf16)
            Va = cst.tile([126, 32], bf16)
            Ds = cst.tile([128, 3], bf16)
            Eb = cst.tile([128, 4 * 126], bf16)
            Rb = cst.tile([3, 126], f32r)
            Wat = cst.tile([126, 128], f32r)
            Wop = cst.tile([126, 128], bf16)
            WH1 = cst.tile([128, 512], f32r)
            WH2 = cst.tile([128, 512], f32r)
            WOUT = cst.tile([128, 8], f32r)
            Bown = cst.tile([126, 1], f32)
            Bint = cst.tile([126, 1], f32)
            Bat = cst.tile([128, 1], f32)
            Bop = cst.tile([128, 1], f32)
            BH1 = cst.tile([128, 2], f32)
            BH2 = cst.tile([128, 2], f32)
            Bout = cst.tile([4, 1], f32)
            for t_sb, t_dr in [(WiA, wia_d), (Wo, wo_d), (Bown, bown_d),
                               (Bint, bint_d), (WiB, wib_d), (Wk, wk_d),
                               (Wq, wq_d), (Va, va_d), (Ds, ds_d)]:
                nc.sync.dma_start(out=t_sb, in_=t_dr[:, :])

            def load_late_consts():
                for t_sb, t_dr in [(Eb, eb_d), (Rb, rb_d),
                                   (Wv, wv_d), (Wat, wat_d), (Wop, wop_d),
                                   (WH1, wh1_d), (WH2, wh2_d), (WOUT, wout_d),
                                   (Bat, bat_d), (Bop, bop_d), (BH1, bh1_d),
                                   (BH2, bh2_d), (Bout, bout_d)]:
                    nc.scalar.dma_start(out=t_sb, in_=t_dr[:, :])

            with nc.allow_low_precision(reason="bf16 intermediates; final accums f32"):
                state = {}

                def load_x(t):
                    bs = t * NB
                    XO = px.tile([OWN_DIM, NB], bf16, tag="xo", name="XO")
                    XA = px.tile([126, NB], bf16, tag="xa", name="XA")
                    XB = px.tile([14, NB], bf16, tag="xb", name="XB")
                    nc.sync.dma_start(out=XO, in_=xo_d[:, bs:bs + NB])
                    nc.sync.dma_start(out=XA, in_=xa_d[:, bs:bs + NB])
                    nc.sync.dma_start(out=XB, in_=xb_d[:, bs:bs + NB])
                    state[t] = {"X": (XO, XA, XB)}

                def e_pre(t):
                    """own embed for tile t (sm bank use #1 of the cycle)"""
                    XO = state[t]["X"][0]
                    PO = psm.tile([128, NB], f32, tag="sm", name="PO")
                    nc.tensor.matmul(PO[0:126, :], Wo, XO)
                    OWN = ph.tile([126, NB], bf16, tag="own", name="OWN")
                    nc.scalar.activation(OWN, PO[0:126, :], AF.Prelu,
                                         bias=Bown, alpha=ALPHA)
                    ZT = pzt.tile([126, N_INTR, NB], bf16, tag="zt", name="ZT")
                    EG = peg.tile([128, 5, NB], bf16, tag="eg", name="EG")
                    st = state[t]
                    st.update({"OWN": OWN, "ZT": ZT, "EG": EG})

                def z_mm(t, c):
                    """embed matmuls for chunk c (n=2c, 2c+1) -> pz pair bank"""
                    st = state[t]
                    XA, XB = st["X"][1], st["X"][2]
                    PZ = ppz.tile([126, 2, NB], f32, tag="pz", name="PZ")
                    for i in range(2):
                        n = 2 * c + i
                        if n < 18:
                            nc.tensor.matmul(PZ[:, i, :],
                                             WiA[:, n * 126:(n + 1) * 126], XA)
                        else:
                            nc.tensor.matmul(PZ[:, i, :],
                                             WiB[:, (n - 18) * 126:(n - 17) * 126], XB)
                    st[("PZ", c)] = PZ

                def z_act(t, c):
                    st = state[t]
                    nc.scalar.activation(st["ZT"][:, 2 * c:2 * c + 2, :],
                                         st.pop(("PZ", c)),
                                         AF.Prelu, bias=Bint, alpha=ALPHA)

                def kq_mm(t, c):
                    st = state[t]
                    ZT, OWN = st["ZT"], st["OWN"]
                    PK = ppk.tile([126, 2, NB], f32, tag="pk", name="PK")
                    for i in range(2):
                        n = 2 * c + i
                        nc.tensor.matmul(PK[:, i, :], Wk, ZT[:, n, :],
                                         start=True, stop=False)
                        nc.tensor.matmul(PK[:, i, :], Wq, OWN,
                                         start=False, stop=True)
                    st[("PK", c)] = PK

                def tanh_act(t, c):
                    st = state[t]
                    EN = pen.tile([126, 2, NB], bf16, tag="en", name="EN")
                    nc.scalar.activation(EN, st.pop(("PK", c)), AF.Tanh)
                    st[("EN", c)] = EN

                def score_mm(t, c):
                    st = state[t]
                    EN = st.pop(("EN", c))
                    for i in range(2):
                        n = 2 * c + i
                        j = n % 4
                        if j == 0:
                            st["PS"] = pps.tile([128, NB], f32, tag="ps", name="PS")
                        nc.tensor.matmul(st["PS"][32 * j:32 * (j + 1), :], Va,
                                         EN[:, i, :], tile_position=(0, 32 * j))

                def exp_act(t, g):
                    st = state[t]
                    nc.scalar.activation(st["EG"][:, g, :], st["PS"], AF.Exp)

                def egsum(t):
                    """denominator pre-sum on gpsimd (SBUF bf16 only)"""
                    st = state[t]
                    EG = st["EG"]
                    s01 = ph.tile([128, NB], bf16, tag="es0", name="ES0")
                    s23 = ph.tile([128, NB], bf16, tag="es1", name="ES1")
                    s03 = ph.tile([128, NB], bf16, tag="es2", name="ES2")
                    EGS = ph.tile([128, NB], bf16, tag="egs", name="EGS")
                    nc.gpsimd.tensor_tensor(out=s01, in0=EG[:, 0, :], in1=EG[:, 1, :], op=ALU.add)
                    nc.gpsimd.tensor_tensor(out=s23, in0=EG[:, 2, :], in1=EG[:, 3, :], op=ALU.add)
                    nc.gpsimd.tensor_tensor(out=s03, in0=s01, in1=s23, op=ALU.add)
                    nc.gpsimd.tensor_tensor(out=EGS, in0=s03, in1=EG[:, 4, :], op=ALU.add)
                    st["EGS"] = EGS

                def denom(t):
                    """softmax denominator + reciprocal, emitted at E-phase
                    tail so the ctx phase never stalls on the reciprocal"""
                    st = state[t]
                    PD = psm.tile([128, NB], f32, tag="sm", name="PD")
                    nc.tensor.matmul(PD[0:3, :], Ds, st["EGS"])
                    RD = ph.tile([3, NB], f32r, tag="rd", name="RD")
                    nc.vector.reciprocal(RD, PD[0:3, :])
                    st["RD"] = RD

                def build_c_steps(t):
                    """ctx + head for tile t, as a list of interleavable steps"""
                    st = state[t]
                    bs = t * NB

                    def c_rbcast():
                        PR = psm.tile([128, NB], f32, tag="sm", name="PR")
                        nc.tensor.matmul(PR[0:126, :], Rb, st["RD"])
                        PRs = ph.tile([126, NB], f32, tag="prs", name="PRs")
                        nc.vector.tensor_scalar_mul(PRs, PR[0:126, :], 1.0)
                        st["PR"] = PRs

                    def mk_cna(n):
                        def s():
                            g, j = n // 4, n % 4
                            PEb = ppb.tile([126, NB], f32, tag="pb", name="PEb")
                            nc.tensor.matmul(PEb, Eb[:, j * 126:(j + 1) * 126],
                                             st["EG"][:, g, :])
                            PN = pnp.tile([126, NB], bf16, tag="pn", name="PN")
                            nc.vector.tensor_tensor(out=PN, in0=PEb,
                                                    in1=st["ZT"][:, n, :], op=ALU.mult)
                            st[("PN", n)] = PN
                        return s

                    def mk_cnb(n):
                        def s():
                            if n == 0:
                                st["CTXV"] = ppc.tile([126, NB], f32, tag="pc",
                                                      name="CTXV")
                            nc.tensor.matmul(st["CTXV"], Wv, st.pop(("PN", n)),
                                             start=(n == 0), stop=(n == 19))
                        return s

                    def c_norm():
                        CTX = ph.tile([126, NB], f32r, tag="ctx", name="CTX")
                        nc.vector.tensor_tensor(out=CTX, in0=st["CTXV"],
                                                in1=st["PR"], op=ALU.mult)
                        st["CTX"] = CTX

                    def c_att():
                        PH = psm.tile([128, NB], f32, tag="sm", name="PH")
                        nc.tensor.matmul(PH, Wat, st["CTX"])
                        st["ATT"] = ph.tile([128, NB], f32r, tag="att", name="ATT")
                        nc.scalar.activation(st["ATT"], PH, AF.Tanh, bias=Bat)

                    def c_ownp():
                        PH = psm.tile([128, NB], f32, tag="sm", name="PH2")
                        nc.tensor.matmul(PH, Wop, st["OWN"])
                        st["OWV"] = ph.tile([128, NB], f32r, tag="owv", name="OWV")
                        nc.scalar.activation(st["OWV"], PH, AF.Tanh, bias=Bop)

                    def mk_h1(mh):
                        def s():
                            PHh = psm.tile([128, NB], f32, tag="sm", name="PHh")
                            nc.tensor.matmul(PHh, WH1[:, mh * 128:(mh + 1) * 128],
                                             st["OWV"], start=True, stop=False)
                            nc.tensor.matmul(PHh, WH1[:, 256 + mh * 128:256 + (mh + 1) * 128],
                                             st["ATT"], start=False, stop=True)
                            st[f"H1{mh}"] = ph.tile([128, NB], f32r, tag=f"h1{mh}", name="H1")
                            nc.scalar.activation(st[f"H1{mh}"], PHh, AF.Prelu,
                                                 bias=BH1[:, mh:mh + 1], alpha=ALPHA)
                        return s

                    def mk_h2(mh):
                        def s():
                            PHh = psm.tile([128, NB], f32, tag="sm", name="PHh2")
                            nc.tensor.matmul(PHh, WH2[:, mh * 128:(mh + 1) * 128],
                                             st["H10"], start=True, stop=False)
                            nc.tensor.matmul(PHh, WH2[:, 256 + mh * 128:256 + (mh + 1) * 128],
                                             st["H11"], start=False, stop=True)
                            st[f"H2{mh}"] = ph.tile([128, NB], f32r, tag=f"h2{mh}", name="H2")
                            nc.scalar.activation(st[f"H2{mh}"], PHh, AF.Prelu,
                                                 bias=BH2[:, mh:mh + 1], alpha=ALPHA)
                        return s

                    def c_out():
                        PO4 = psm.tile([128, NB], f32, tag="sm", name="PO4")
                        nc.tensor.matmul(PO4[0:4, :], WOUT[:, 0:4], st["H20"],
                                         start=True, stop=False)
                        nc.tensor.matmul(PO4[0:4, :], WOUT[:, 4:8], st["H21"],
                                         start=False, stop=True)
                        OT = ph.tile([4, NB], f32, tag="ot", name="OT")
                        nc.vector.tensor_scalar_add(OT, PO4[0:4, :], Bout)
                        nc.sync.dma_start(out=out_d[:, bs:bs + NB], in_=OT)
                        del state[t]

                    cns = [c_rbcast, mk_cna(0), mk_cna(1)]
                    for n in range(2, N_INTR):
                        cns += [mk_cnb(n - 2), mk_cna(n)]
                    cns += [mk_cnb(N_INTR - 2), mk_cnb(N_INTR - 1)]
                    return (cns + [c_norm, c_att, c_ownp, mk_h1(0), mk_h1(1),
                                   mk_h2(0), mk_h2(1), c_out])

                def emit_tile(t, csteps):
                    """stage E of tile t interleaved with ctx/head steps of t-1.

                    3-stage stagger inside E: z matmuls of chunk c, k+q of c-1,
                    scores of c-2 — keeps PE fed while scalar drains PSUM."""
                    ci = 0

                    def c_run(k):
                        nonlocal ci
                        for _ in range(k):
                            if ci < len(csteps):
                                csteps[ci]()
                                ci += 1

                    e_pre(t)
                    NCH = N_INTR // 2
                    for c in range(NCH + 4):
                        if c < NCH:
                            z_mm(t, c)
                        c_run(2)
                        if 1 <= c < NCH + 1:
                            z_act(t, c - 1)
                        if 2 <= c < NCH + 2:
                            kq_mm(t, c - 2)
                        c_run(2)
                        if 3 <= c < NCH + 3:
                            tanh_act(t, c - 3)
                        if 4 <= c < NCH + 4:
                            score_mm(t, c - 4)
                            if (c - 4) % 2 == 1:
                                exp_act(t, (c - 4) // 2)
                    egsum(t)
                    denom(t)
                    c_run(len(csteps))

                # ---- software pipeline over tiles ----
                load_x(0)
                load_x(1)
                emit_tile(0, [])
                load_late_consts()
                for t in range(1, NT):
                    if t + 1 < NT:
                        load_x(t + 1)
                    emit_tile(t, build_c_steps(t - 1))
                for s in build_c_steps(NT - 1):
                    s()

    nc.compile()
    return nc


def _host_prep(inputs):
    """Build per-core input maps (numpy only)."""
    obs = np.ascontiguousarray(inputs["obs"], dtype=np.float32)
    w_own = np.asarray(inputs["w_own"], np.float32)
    w_int = np.asarray(inputs["w_int"], np.float32)
    wq = np.asarray(inputs["wq"], np.float32)
    wk = np.asarray(inputs["wk"], np.float32)
    wv = np.asarray(inputs["wv"], np.float32)
    v_att = np.asarray(inputs["v_att"], np.float32)
    w_attn = np.asarray(inputs["w_attn"], np.float32)
    w_ownp = np.asarray(inputs["w_ownp"], np.float32)
    w_h1 = np.asarray(inputs["w_h1"], np.float32)
    w_h2 = np.asarray(inputs["w_h2"], np.float32)
    w_out = np.asarray(inputs["w_out"], np.float32)

    def blockdiag(w):  # [H, D, D] -> [126, 126]
        out = np.zeros((TOT, TOT), np.float32)
        for h in range(H):
            out[h * D:(h + 1) * D, h * D:(h + 1) * D] = w[h]
        return out

    wia = np.zeros((126, 18 * 126), np.float32)
    for n in range(18):
        wia[7 * n:7 * n + 7, n * 126:(n + 1) * 126] = w_int
    wib = np.zeros((14, 2 * 126), np.float32)
    for n in range(2):
        wib[7 * n:7 * n + 7, n * 126:(n + 1) * 126] = w_int

    va32 = np.zeros((126, 32), np.float32)
    for h in range(H):
        va32[h * D:(h + 1) * D, h] = v_att[h]

    densel = np.zeros((128, 3), np.float32)
    for j in range(4):
        for h in range(H):
            densel[32 * j + h, h] = 1.0

    ebcsel = np.zeros((128, 4 * 126), np.float32)
    for j in range(4):
        for h in range(H):
            ebcsel[32 * j + h, j * 126 + h * D:(j * 126) + (h + 1) * D] = 1.0

    rbc = np.zeros((3, 126), np.float32)
    for h in range(H):
        rbc[h, h * D:(h + 1) * D] = 1.0

    wh1r = np.ascontiguousarray(
        w_h1.reshape(2, 128, HID).transpose(1, 0, 2).reshape(128, 512))
    wh2r = np.ascontiguousarray(
        w_h2.reshape(2, 128, HID).transpose(1, 0, 2).reshape(128, 512))
    woutr = np.ascontiguousarray(
        w_out.reshape(2, 128, NOUT).transpose(1, 0, 2).reshape(128, 8))

    params = {
        "wia": wia.astype(_bf16np), "wib": wib.astype(_bf16np), "wo": w_own.astype(_bf16np),
        "wqb": blockdiag(wq).astype(_bf16np), "wkb": blockdiag(wk).astype(_bf16np), "wvb": blockdiag(wv).astype(_bf16np),
        "va32": va32.astype(_bf16np), "densel": densel.astype(_bf16np), "ebcsel": ebcsel.astype(_bf16np), "rbc": rbc,
        "wat": w_attn, "wop": w_ownp.astype(_bf16np),
        "wh1r": wh1r, "wh2r": wh2r, "woutr": woutr,
        "bown": np.asarray(inputs["b_own"], np.float32).reshape(126, 1),
        "bint": np.asarray(inputs["b_int"], np.float32).reshape(126, 1),
        "bat": np.asarray(inputs["b_attn"], np.float32).reshape(128, 1),
        "bop": np.asarray(inputs["b_ownp"], np.float32).reshape(128, 1),
        "bh1": np.ascontiguousarray(
            np.asarray(inputs["b_h1"], np.float32).reshape(2, 128).T),
        "bh2": np.ascontiguousarray(
            np.asarray(inputs["b_h2"], np.float32).reshape(2, 128).T),
        "bout": np.asarray(inputs["b_out"], np.float32).reshape(4, 1),
    }

    in_maps = []
    for c in range(N_CORES):
        sl = obs[c * BC:(c + 1) * BC]                       # [BC, 147]
        xo = np.ascontiguousarray(sl[:, :OWN_DIM].T).astype(_bf16np)        # [7, BC]
        intr = sl[:, OWN_DIM:].reshape(BC, N_INTR, INT_DIM)  # [BC, 20, 7]
        intrT = intr.transpose(1, 2, 0)                     # [20, 7, BC]
        xa = np.ascontiguousarray(intrT[:18].reshape(126, BC)).astype(_bf16np)
        xb = np.ascontiguousarray(intrT[18:].reshape(14, BC)).astype(_bf16np)
        m = {"xo": xo, "xa": xa, "xb": xb}
        m.update(params)
        in_maps.append(m)
    return in_maps


def _get_nc():
    if "nc" not in _BUILT:
        _BUILT["nc"] = _build_nc()
    return _BUILT["nc"]


def run(inputs, trace=False):
    from concourse.bass_utils import run_bass_kernel_spmd
    nc = _get_nc()
    in_maps = _host_prep(inputs)
    res = run_bass_kernel_spmd(nc, in_maps, core_ids=list(range(N_CORES)),
                               trace=trace)
    outs = [res.results[c]["outT"] for c in range(N_CORES)]   # each [4, BC]
    full = np.concatenate(outs, axis=1).T                     # [B, 4]
    return np.ascontiguousarray(full, dtype=np.float32), res


def kernel(**inputs):
    out, _ = run(inputs, trace=False)
    return out
